# revision 15
# baseline (speedup 1.0000x reference)
"""GRU encoder (nn_BidirectionalLemmaEncoder) Trainium2 Bass kernel.

Strategy: shard the recurrence 8-way over the hidden dimension. Core c owns
hidden units [128c, 128c+128) i.e. 384 gate rows of W_hh/W_ih. Every core
keeps the full batch (32). Each step:
  - PE: psum_rz[g,b] = sum_k W_rz[:,k-blk]^T @ h_blk + gi via one-hot matmul
        psum_n = gh_n (+b_hh_n via ones-trick) ; psum_gin = gi_n
  - ACT: sigmoid on [128,64] (r|z), later tanh
  - DVE: n = tanh(gi_n + r*gh_n); h' = (1-z)*n + z*h
  - remote_dma_broadcast: each core sends its new h chunk [128,32] (bf16) to
    the 7 peers with XOR-relative addressing; host pre-permutes each core's
    W K-chunks so slot m always holds h-block (c XOR m).
The input-side GEMM collapses to gi_vocab = emb @ W_ih_shard.T (VOCAB=256)
computed once on device; per-step gi comes in through one-hot matmuls that
accumulate straight into the gate PSUM (bias terms folded into gi_vocab).
"""

import os
from contextlib import ExitStack

import numpy as np
import ml_dtypes

import concourse.bass as bass
import concourse.bacc as bacc
import concourse.mybir as mybir
from concourse.bass_utils import run_bass_kernel_spmd

BATCH = 32
SEQ = int(os.environ.get("BASS_GRU_S", "512"))
HID = 1024
VOCAB = 256
NSH = 8          # cores / hidden shards
HC = HID // NSH  # 128 hidden units per core
GC = 3 * HC      # 384 gate rows per core
N = BATCH        # free dim of the recurrent matmul
OH_DEPTH = 8     # one-hot prefetch depth (must divide nothing special, >=4)

BF16 = mybir.dt.bfloat16
F32 = mybir.dt.float32

_cache = {}


def build_kernel(seq=SEQ, warp_comp=True):
    """Build the SPMD bass program (same program for all 8 cores)."""
    nc = bacc.Bacc()

    # ---- DRAM I/O ----
    whh_ext = nc.declare_dram_parameter("whh", [HC, 3, NSH, HC], BF16, isOutput=False)
    embT_ext = nc.declare_dram_parameter("embT", [NSH, HC, VOCAB], F32, isOutput=False)
    wih_ext = nc.declare_dram_parameter("wih", [NSH, HC, GC], F32, isOutput=False)
    fold_ext = nc.declare_dram_parameter("fold", [128, GC], F32, isOutput=False)
    biasn_ext = nc.declare_dram_parameter("biasn", [1, HC], BF16, isOutput=False)
    oh_ext = nc.declare_dram_parameter("oh", [seq, VOCAB // 2, 2 * N], BF16, isOutput=False)
    ann_ext = nc.declare_dram_parameter("ann", [seq, HC, N], BF16, isOutput=True)

    with ExitStack() as stack:
        ec = stack.enter_context
        # ---- SBUF ----
        sb_whh = ec(nc.sbuf_tensor("sb_whh", [HC, 3, NSH, HC], BF16))
        sb_embT = ec(nc.sbuf_tensor("sb_embT", [HC, NSH, VOCAB], F32))
        sb_wih = ec(nc.sbuf_tensor("sb_wih", [HC, NSH, GC], F32))
        sb_fold = ec(nc.sbuf_tensor("sb_fold", [128, GC], F32))
        sb_biasn = ec(nc.sbuf_tensor("sb_biasn", [1, HC], BF16))
        sb_ones = ec(nc.sbuf_tensor("sb_ones", [1, N], BF16))
        sb_givoc = ec(nc.sbuf_tensor("sb_givoc", [128, 2, GC], BF16))
        ring0 = ec(nc.sbuf_tensor("ring0", [128, NSH, N], BF16))
        ring1 = ec(nc.sbuf_tensor("ring1", [128, NSH, N], BF16))
        sb_h32 = ec(nc.sbuf_tensor("sb_h32", [128, N], F32))
        sb_oh = ec(nc.sbuf_tensor("sb_oh", [128, OH_DEPTH, 2 * N], BF16))
        sb_rz = ec(nc.sbuf_tensor("sb_rz", [128, 2 * N], F32))
        sb_t = ec(nc.sbuf_tensor("sb_t", [128, N], F32))
        sb_t2 = ec(nc.sbuf_tensor("sb_t2", [128, N], F32))
        sb_omz = ec(nc.sbuf_tensor("sb_omz", [128, N], F32))
        sb_zh = ec(nc.sbuf_tensor("sb_zh", [128, N], F32))
        sb_nn = ec(nc.sbuf_tensor("sb_nn", [128, N], F32))
        sb_u = ec(nc.sbuf_tensor("sb_u", [128, N], F32))
        # ---- PSUM (full banks to guarantee bank separation) ----
        ps_rz0 = ec(nc.psum_tensor("ps_rz0", [128, 512], F32))
        ps_rz1 = ec(nc.psum_tensor("ps_rz1", [128, 512], F32))
        ps_n0 = ec(nc.psum_tensor("ps_n0", [128, 512], F32))
        ps_n1 = ec(nc.psum_tensor("ps_n1", [128, 512], F32))
        ps_gv0 = ec(nc.psum_tensor("ps_gv0", [128, 512], F32))
        ps_gv1 = ec(nc.psum_tensor("ps_gv1", [128, 512], F32))
        # ---- semaphores ----
        dma_sem = ec(nc.semaphore("dma_sem"))
        oh_slots = [ec(nc.semaphore(f"oh_s{i}")) for i in range(OH_DEPTH)]
        ann_semP = [ec(nc.semaphore(f"ann_p{i}")) for i in range(2)]
        prep_sem = ec(nc.semaphore("prep_sem"))
        local_semP = [ec(nc.semaphore(f"local_p{i}")) for i in range(2)]
        remote_semPM = [[ec(nc.semaphore(f"remote_p{i}m{m}")) for m in range(NSH)] for i in range(2)]
        bar_sem = ec(nc.semaphore("bar_sem"))
        bar_lsem = ec(nc.semaphore("bar_lsem"))
        v_init = ec(nc.semaphore("v_init"))
        mm_gv = ec(nc.semaphore("mm_gv"))
        v_givoc = ec(nc.semaphore("v_givoc"))
        sem_h = ec(nc.semaphore("sem_h"))
        sem_r = ec(nc.semaphore("sem_r"))
        sem_z = ec(nc.semaphore("sem_z"))
        sem_n = ec(nc.semaphore("sem_n"))
        sem_sig_r = ec(nc.semaphore("sem_sig_r"))
        sem_sig_z = ec(nc.semaphore("sem_sig_z"))
        sem_t2 = ec(nc.semaphore("sem_t2"))
        sem_tanh = ec(nc.semaphore("sem_tanh"))
        block = ec(nc.Block())
        rings = [ring0, ring1]
        ps_rz = [ps_rz0, ps_rz1]
        ps_n = [ps_n0, ps_n1]
        ps_gv = [ps_gv0, ps_gv1]

        # number of init DMAs on dma_sem, in issue order:
        # whh(1), embT(8), wih(8), fold(1), biasn(1)
        N_INIT_DMA = 1 + NSH + NSH + 1 + 1
        DMA_ALL = 16 * N_INIT_DMA

        @block.sync
        def _(sync):
            sync.dma_start(out=sb_whh[:, :, :, :], in_=whh_ext[:, :, :, :]).then_inc(dma_sem, 16)
            for kc in range(NSH):
                sync.dma_start(out=sb_embT[:, kc, :], in_=embT_ext[kc, :, :]).then_inc(dma_sem, 16)
            for kc in range(NSH):
                sync.dma_start(out=sb_wih[:, kc, :], in_=wih_ext[kc, :, :]).then_inc(dma_sem, 16)
            sync.dma_start(out=sb_fold[:, :], in_=fold_ext[:, :]).then_inc(dma_sem, 16)
            sync.dma_start(out=sb_biasn[:, :], in_=biasn_ext[:, :]).then_inc(dma_sem, 16)
            # initial one-hot prefetch
            for t in range(min(OH_DEPTH, seq)):
                sync.dma_start(out=sb_oh[:, t % OH_DEPTH, :], in_=oh_ext[t, :, :]).then_inc(oh_slots[t % OH_DEPTH], 16)
            # steady state
            for t in range(seq):
                p1 = (t + 1) & 1
                # prefetch one-hot for t+OH_DEPTH once PE consumed oh(t)
                if t + OH_DEPTH < seq:
                    sync.wait_ge(sem_n, t + 1)
                    sync.dma_start(
                        out=sb_oh[:, (t + OH_DEPTH) % OH_DEPTH, :], in_=oh_ext[t + OH_DEPTH, :, :]
                    ).then_inc(oh_slots[t % OH_DEPTH], 16)
                # annotation output h(t+1) = ring[p1][slot0]
                sync.wait_ge(sem_h, t + 1)
                sync.dma_start(out=ann_ext[t, :, :], in_=rings[p1][:, 0, :]).then_inc(ann_semP[p1], 16)
            sync.wait_ge(ann_semP[0], 16 * (seq // 2))
            sync.wait_ge(ann_semP[1], 16 * ((seq + 1) // 2))

        @block.vector
        def _(vector):
            vector.memset(ring0[:, :, :], 0.0).then_inc(v_init, 1)
            vector.memset(ring1[:, :, :], 0.0).then_inc(v_init, 1)
            vector.memset(sb_h32[:, :], 0.0).then_inc(v_init, 1)
            vector.memset(sb_ones[:, :], 1.0).then_inc(v_init, 1)
            # gi_vocab = emb @ W_ih_sh^T + fold, cast to bf16
            vector.wait_ge(dma_sem, DMA_ALL)
            vector.wait_ge(mm_gv, 2)
            for vc in range(2):
                vector.tensor_add(sb_givoc[:, vc, :], ps_gv[vc][:, 0:GC], sb_fold[:, :]).then_inc(v_givoc, 1)
            for t in range(seq):
                p = t & 1
                p1 = (t + 1) & 1
                vector.wait_ge(sem_sig_r, t + 1)
                vector.wait_ge(sem_n, t + 1)
                # t = r * (gh_n + b_hh_n)   (bias already in psum via ones-MM)
                vector.tensor_mul(sb_t[:, :], sb_rz[:, 0:N], ps_n[p][:, 0:N])
                vector.drain()
                # t2 = t + gi_n
                vector.tensor_add(sb_t2[:, :], sb_t[:, :], ps_n[p][:, N:2 * N]).then_inc(sem_t2, 1)
                # omz = 1 - z ; zh = z * h_prev   (overlap ACT tanh)
                vector.wait_ge(sem_sig_z, t + 1)
                vector.tensor_scalar(sb_omz[:, :], sb_rz[:, N:2 * N], -1.0, 1.0,
                                     mybir.AluOpType.mult, mybir.AluOpType.add)
                vector.tensor_mul(sb_zh[:, :], sb_rz[:, N:2 * N], sb_h32[:, :])
                vector.wait_ge(sem_tanh, t + 1)
                vector.tensor_mul(sb_u[:, :], sb_omz[:, :], sb_nn[:, :])
                vector.drain()
                # h_new -> ring slot 0 (bf16) ; guard slot reuse (sends of t-2, ann dma of t-2)
                if t >= 2:
                    vector.wait_ge(local_semP[p1], 112 * (t // 2))
                    vector.wait_ge(ann_semP[p1], 16 * (t // 2))
                vector.tensor_add(rings[p1][:, 0, :], sb_u[:, :], sb_zh[:, :]).then_inc(sem_h, 1)
                vector.tensor_add(sb_h32[:, :], sb_u[:, :], sb_zh[:, :])

        @block.tensor
        def _(tensor):
            # one-time gi_vocab GEMM: out[v, g] = sum_h emb[v,h] * W_ih_sh[g,h]
            tensor.wait_ge(dma_sem, DMA_ALL)
            for vc in range(2):
                for kc in range(NSH):
                    inst = tensor.matmul(
                        ps_gv[vc][:, 0:GC],
                        sb_embT[:, kc, 128 * vc:128 * (vc + 1)],
                        sb_wih[:, kc, :],
                        start=(kc == 0), stop=(kc == NSH - 1),
                    )
                    if kc == NSH - 1:
                        inst.then_inc(mm_gv, 1)
            tensor.wait_ge(v_givoc, 2)
            tensor.wait_ge(v_init, 4)            # rings zeroed, ones set
            for t in range(seq):
                p = t & 1
                prz = ps_rz[p]
                pn = ps_n[p]
                if t >= 1:
                    tensor.wait_ge(sem_h, t)
                tensor.wait_ge(oh_slots[t % OH_DEPTH], 16 * (t // OH_DEPTH + 1))
                if t >= 2:
                    tensor.wait_ge(sem_sig_r, t - 1)
                    tensor.wait_ge(sem_sig_z, t - 1)
                    tensor.wait_ge(sem_t2, t - 1)
                oh = sb_oh[:, t % OH_DEPTH, :]
                rg = rings[p]
                nw = (t + 1) // 2
                # --- r phase (start=True clears the rz bank) ---
                tensor.matmul(prz[:, 0:N], sb_whh[:, 0, 0, :], rg[:, 0, :], start=True, stop=False)
                for vc in range(2):
                    ohc = oh[:, vc * N:(vc + 1) * N]
                    tensor.matmul(prz[:, 0:N], sb_givoc[:, vc, 0:128], ohc, start=False, stop=False)
                for m in range(1, NSH):
                    last = m == NSH - 1
                    if t >= 1:
                        tensor.wait_ge(remote_semPM[t & 1][m], 2 * nw)
                    inst = tensor.matmul(prz[:, 0:N], sb_whh[:, 0, m, :], rg[:, m, :], start=False, stop=last)
                    if last:
                        inst.then_inc(sem_r, 1)
                # --- n phase (bias-ones clears the n bank) ---
                tensor.matmul(pn[:, 0:N], sb_biasn[0:1, :], sb_ones[0:1, :], start=True, stop=False)
                tensor.matmul(pn[:, 0:N], sb_whh[:, 2, 0, :], rg[:, 0, :], start=False, stop=False)
                for vc in range(2):
                    ohc = oh[:, vc * N:(vc + 1) * N]
                    tensor.matmul(pn[:, N:2 * N], sb_givoc[:, vc, 256:384], ohc, start=False, stop=False)
                for m in range(1, NSH):
                    last = m == NSH - 1
                    inst = tensor.matmul(pn[:, 0:N], sb_whh[:, 2, m, :], rg[:, m, :], start=False, stop=last)
                    if last:
                        inst.then_inc(sem_n, 1)
                # --- z phase (overlaps the DVE n-chain) ---
                tensor.matmul(prz[:, N:2 * N], sb_whh[:, 1, 0, :], rg[:, 0, :], start=False, stop=False)
                for vc in range(2):
                    ohc = oh[:, vc * N:(vc + 1) * N]
                    tensor.matmul(prz[:, N:2 * N], sb_givoc[:, vc, 128:256], ohc, start=False, stop=False)
                for m in range(1, NSH):
                    last = m == NSH - 1
                    inst = tensor.matmul(prz[:, N:2 * N], sb_whh[:, 1, m, :], rg[:, m, :], start=False, stop=last)
                    if last:
                        inst.then_inc(sem_z, 1)

        @block.scalar
        def _(scalar):
            for t in range(seq):
                p = t & 1
                scalar.wait_ge(sem_r, t + 1)
                scalar.activation(sb_rz[:, 0:N], ps_rz[p][:, 0:N],
                                  mybir.ActivationFunctionType.Sigmoid).then_inc(sem_sig_r, 1)
                scalar.wait_ge(sem_z, t + 1)
                scalar.activation(sb_rz[:, N:2 * N], ps_rz[p][:, N:2 * N],
                                  mybir.ActivationFunctionType.Sigmoid).then_inc(sem_sig_z, 1)
                scalar.wait_ge(sem_t2, t + 1)
                scalar.activation(sb_nn[:, :], sb_t2[:, :],
                                  mybir.ActivationFunctionType.Tanh).then_inc(sem_tanh, 1)

        @block.gpsimd
        def _(gpsimd):
            # barrier: every core's rings are zeroed before any peer sends
            gpsimd.wait_ge(v_init, 4)
            gpsimd.remote_sem_update_broadcast(
                remote_sem=bar_sem, local_sem=bar_lsem,
                rdests=[None] + [(0, k) for k in range(1, NSH)],
            ).then_inc(prep_sem, 1)
            gpsimd.wait_ge(prep_sem, 1)
            gpsimd.trigger_dma(count=1)
            gpsimd.wait_ge(bar_sem, 2 * (NSH - 1))
            for t in range(seq - 1):      # last step needs no broadcast
                p1 = (t + 1) & 1
                rg1 = rings[p1]
                for m in range(1, NSH):
                    delta = (m ^ 2) if (warp_comp and m >= 4) else m
                    rdests = [None] * NSH
                    rdests[delta] = (0, delta)
                    gpsimd.remote_dma_broadcast(
                        out_ap=rg1[:, m, :], in_ap=rg1[:, 0, :],
                        remote_sem=remote_semPM[p1][m], local_sem=local_semP[p1],
                        rdests=rdests,
                    ).then_inc(prep_sem, 1)
                gpsimd.wait_ge(prep_sem, 1 + 7 * (t + 1))
                gpsimd.wait_ge(sem_h, t + 1)
                gpsimd.trigger_dma(count=7)
            n_even = len([k for k in range(seq - 1) if (k + 1) & 1 == 1])
            n_odd = (seq - 1) - n_even
            gpsimd.wait_ge(local_semP[1], 112 * n_even)
            gpsimd.wait_ge(local_semP[0], 112 * n_odd)

    nc.finalize()
    return nc


def _host_prep(x, emb, W_ih, W_hh, b_ih, b_hh, seq):
    """Build per-core input dicts."""
    bf16 = ml_dtypes.bfloat16
    emb = np.asarray(emb, np.float32)
    W_ih = np.asarray(W_ih, np.float32)
    W_hh = np.asarray(W_hh, np.float32)
    b_ih = np.asarray(b_ih, np.float32)
    b_hh = np.asarray(b_hh, np.float32)
    x = np.asarray(x)

    embT = np.ascontiguousarray(emb.T.reshape(NSH, HC, VOCAB))

    # one-hot: oh[t, p, vc*N + b] = (x[b, t] == 128*vc + p)
    ohf = np.zeros((seq, VOCAB, BATCH), np.float32)
    t_idx = np.arange(seq)[:, None]
    b_idx = np.arange(BATCH)[None, :]
    ohf[t_idx, x.T[:seq], b_idx] = 1.0
    oh = np.ascontiguousarray(
        ohf.reshape(seq, 2, HC, BATCH).transpose(0, 2, 1, 3).reshape(seq, HC, 2 * N)
    ).astype(bf16)

    in_maps = []
    for c in range(NSH):
        rows = lambda g: slice(g * HID + c * HC, g * HID + (c + 1) * HC)
        whh = np.empty((HC, 3, NSH, HC), np.float32)
        for g in range(3):
            Wg = W_hh[rows(g), :]
            for m in range(NSH):
                blk = c ^ m
                whh[:, g, m, :] = Wg[:, blk * HC:(blk + 1) * HC].T
        Wsh = np.concatenate([W_ih[rows(g), :] for g in range(3)], 0)  # [384, 1024]
        wih = np.ascontiguousarray(Wsh.T.reshape(NSH, HC, GC))
        fv = np.concatenate([
            b_ih[rows(0)] + b_hh[rows(0)],
            b_ih[rows(1)] + b_hh[rows(1)],
            b_ih[rows(2)],
        ]).astype(np.float32)
        fold = np.ascontiguousarray(np.broadcast_to(fv, (128, GC)))
        biasn = b_hh[rows(2)].reshape(1, HC)
        in_maps.append({
            "whh": whh.astype(bf16),
            "embT": embT,
            "wih": wih,
            "fold": fold,
            "biasn": biasn.astype(bf16),
            "oh": oh,
        })
    return in_maps


def kernel(x, src_lengths, embedding, W_ih, W_hh, b_ih, b_hh):
    seq = SEQ
    key = ("nc", seq)
    if key not in _cache:
        _cache[key] = build_kernel(seq=seq, warp_comp=True)
    nc = _cache[key]

    in_maps = _host_prep(x, embedding, W_ih, W_hh, b_ih, b_hh, seq)
    res = run_bass_kernel_spmd(nc, in_maps, list(range(NSH)))

    ann = np.empty((BATCH, seq, HID), np.float32)
    for c in range(NSH):
        a = np.asarray(res.results[c]["ann"]).astype(np.float32)  # [seq, HC, N]
        ann[:, :, c * HC:(c + 1) * HC] = a.transpose(2, 0, 1)
    sl = np.asarray(src_lengths).astype(np.int64)
    final_hidden = ann[np.arange(BATCH), sl]
    return ann, final_hidden
